# revision 27
# baseline (speedup 1.0000x reference)
"""Trainium2 Bass kernel for nn_MultiHeadAttention (B=4, S=2048, DIM=768,
EMBED=512, HEADS=8, HEAD_DIM=64), distributed over 8 NeuronCores.

Sharding: core (b, g) with b in 0..3 (batch, data parallel) and g in 0..1
(head-group of 4 heads, tensor parallel). Each core computes a partial
output Y_partial[b,g] = softmax(QK^T/8) V @ Wo[g-slice]; the host sums the
two group partials per batch and adds the fused output bias bv@wo + bo
(bv is removable on device because softmax rows sum to 1; bk only shifts
each softmax row by a per-query constant, so it is dropped entirely).

Device dataflow per core (bf16 matmuls, fp32 PSUM accumulation):
  - host supplies x^T (DIM on partitions) so no on-device transposes exist
  - Q^T = Wq^T x^T + bq, K^T = Wk^T x^T (no bias) -> [256, S]
  - V packed as [V_h | ones] / [ones | V_h] per local head so the PV
    matmul also emits the softmax denominator replicated 64x
  - S^T via lhsT=K^T chunk, rhs=Q^T block; two heads run as concurrent
    K=64 PE tiles (row packing)
  - exp on ScalarE ACTIVATE(Exp, scale=1/8) straight out of PSUM, FD=1024;
    the first N_DVE chunks of each block run on VectorE instead via a
    Schraudolph bit-exp (int16(s*A+B) bitcast to bf16, ~2-4% sawtooth
    that mostly cancels in the softmax ratio)
  - 1/rowsum via 2-step Newton from a constant seed on GpSimd (all-SBUF),
    keeping VectorE free for PSUM evacuations
  - O^T.T @ Wo -> [S, DIM], evac + DMA per s-chunk
Scheduling: activations stream in [128,512] pieces split across both DMA
rings in consumption order (xk-n0, xq-n0, xk-n1, xv-n0, xv-n1, xk-n2/3,
xv-n2/3, xq-n1..3) so the first exp fires ~8us in; K/V projections and the
block-0 PV backlog (V lands late) drain through a per-chunk filler schedule
inside the attention loops so the PE stream never head-blocks on a DMA.
A post-pass splits multi-semaphore waits and the gpsimd RANGE_CLEAR into
single-wait NoOps for this image's stricter walrus.
"""

import numpy as np
import ml_dtypes

import concourse.bass as bass
import concourse.tile as tile
from concourse import mybir
from concourse.bass_utils import run_bass_kernel_spmd

BF16 = mybir.dt.bfloat16
F32 = mybir.dt.float32
I16 = mybir.dt.int16
NPBF16 = ml_dtypes.bfloat16

B, S, DIM, EMBED, HEADS, HEAD_DIM = 4, 2048, 768, 512, 8, 64
P = 128
KD = DIM // P          # 6   contraction chunks for projections
GROUPS = 2             # head-groups (tensor-parallel split)
GE = EMBED // GROUPS   # 256 embed columns per group
GH = HEADS // GROUPS   # 4   heads per group
MQ = GE // P           # 2   e-chunks per group
SC = S // P            # 16  sequence chunks of 128
NB = 512               # matmul free-dim block
NQ = S // NB           # 4   query blocks
SCALE = 0.125          # 1/sqrt(HEAD_DIM)
NCORES = B * GROUPS    # 8
X0 = 1.0 / 2146.0      # Newton seed for 1/rowsum

# Schraudolph bit-exp on DVE for the first N_DVE chunks of each block:
# bf16 bits of exp(s*SCALE) ~ round(s * 128*log2(e)*SCALE + 127*128 - sig)
N_DVE = 2
SCH_A = 128.0 * 1.4426950408889634 * SCALE
SCH_B = 127.0 * 128.0 - 0.5 * 128.0 * 0.043

ESP_BUFS = 16          # es backlog depth (chunks) for the deferred-PV ramp
PV_DRAIN_FROM = 12     # global chunk index where block-0 PV drains start


def _split_multi_waits(nc):
    """The walrus build in this image accepts at most ONE sem-wait per
    instruction (setupSyncWait: 'Too many sync wait commands'), while Tile
    freely attaches several.  Hoist all but the last wait of each
    instruction onto same-engine NoOps inserted immediately before it —
    identical blocking semantics, one wait per instruction."""
    ctr = 0
    for f in nc.m.functions:
        for blk in f.blocks:
            il = blk.instructions
            out = []
            for inst in il:
                if type(inst).__name__ == "InstISA":
                    # kernel-tail gpsimd.sem_clear (RANGE_CLEAR): this
                    # walrus rejects its encoding ("ISA wrong length").
                    # NRT re-initializes semaphore state per execution, so
                    # replace it with a NoOp carrying the same syncs.
                    nop = mybir.InstNoOp(
                        name=f"{inst.name}-isanop", ins=[], outs=[]
                    )
                    nop.engine = inst.engine
                    nop.sync_info = inst.sync_info
                    out.append(nop)
                    continue
                si = inst.sync_info
                if si is not None and si.on_wait and len(si.on_wait) > 1:
                    waits = list(si.on_wait)
                    for w in waits[:-1]:
                        ctr += 1
                        nop = mybir.InstNoOp(
                            name=f"I-waitsplit-{ctr}", ins=[], outs=[]
                        )
                        nop.engine = inst.engine
                        nop.sync_info = mybir.SyncInfo(on_wait=[w], on_update=[])
                        out.append(nop)
                    si.on_wait = [waits[-1]]
                out.append(inst)
            il[:] = out
    return ctr


def build_nc(split_waits=True):
    nc = bass.Bass("TRN2", target_bir_lowering=False, debug=False)

    xqT = nc.dram_tensor("xqT", [DIM, S], BF16, kind="ExternalInput").ap()
    xkT = nc.dram_tensor("xkT", [DIM, S], BF16, kind="ExternalInput").ap()
    xvT = nc.dram_tensor("xvT", [DIM, S], BF16, kind="ExternalInput").ap()
    wq = nc.dram_tensor("wq", [DIM, GE], BF16, kind="ExternalInput").ap()
    wk = nc.dram_tensor("wk", [DIM, GE], BF16, kind="ExternalInput").ap()
    wv = nc.dram_tensor("wv", [DIM, GE], BF16, kind="ExternalInput").ap()
    wo = nc.dram_tensor("wo", [GE, DIM], BF16, kind="ExternalInput").ap()
    bq = nc.dram_tensor("bq", [GE], F32, kind="ExternalInput").ap()
    out = nc.dram_tensor("out", [S, DIM], F32, kind="ExternalOutput").ap()

    add = mybir.AluOpType.add
    mult = mybir.AluOpType.mult
    Exp = mybir.ActivationFunctionType.Exp

    with tile.TileContext(nc) as tc:
        with (
            tc.tile_pool(name="const", bufs=1) as const,
            # PSUM: "s" = 2 slots x [P,2,NB] (score pairs, 4 banks);
            #       "u" = 4 slots x 1 bank (proj blocks, PV accumulators,
            #             next-q proj, out-proj halves) = 8 banks total.
            tc.tile_pool(name="psS", bufs=2, space="PSUM") as psS,
            tc.tile_pool(name="psU", bufs=4, space="PSUM") as psU,
            tc.tile_pool(name="esp", bufs=ESP_BUFS) as esp,
            tc.tile_pool(name="nrm", bufs=2) as nrm,
            tc.tile_pool(name="yout", bufs=3) as yout,
            tc.tile_pool(name="xin", bufs=3) as xin,
        ):
            wq_sb = const.tile([P, KD, GE], BF16, tag="wq")
            wk_sb = const.tile([P, KD, GE], BF16, tag="wk")
            wv_sb = const.tile([P, KD, GE], BF16, tag="wv")
            wo_sb = const.tile([P, MQ, DIM], BF16, tag="wo")
            bq_sb = const.tile([P, MQ], F32, tag="bq")
            qt_sb = const.tile([P, MQ, S], BF16, tag="qt")   # Q^T
            kt_sb = const.tile([P, MQ, S], BF16, tag="kt")   # K^T
            ot_sb = const.tile([P, MQ, S], BF16, tag="ot")   # O^T
            # V in PV-lhsT layout: per (s-chunk, head) a [128, 128] block
            # of [V_h | ones] (even local head) or [ones | V_h] (odd); the
            # ones columns make the PV matmul also produce the softmax
            # denominator (replicated 64x), partition-aligned with the
            # other head's slot.
            v_sb = const.tile([P, SC, GH, P], BF16, tag="v")
            nc.vector.memset(v_sb[:], 1.0)

            # small ones tile: PE warm-up operand (HAM un-throttles after
            # ~3.4us of sustained matmul activity; warming on dummies while
            # the first DMAs stream means the real prelude runs at 2.4GHz).
            # memset on gpsimd: the DVE is busy with the big v_sb memset.
            dum_sb = const.tile([P, NB], BF16, tag="dum")
            nc.gpsimd.memset(dum_sb[:], 1.0)

            # --- DMAs: sync ring carries weights + xq n-block 0 (so the
            # Q0 projection starts early); gpsimd ring carries the rest in
            # [128,1024] halves (DMA triggers cost ~0.64us each, so pieces
            # stay coarse), ordered by first consumption.
            xk_sb = xin.tile([P, KD, S], BF16, tag="x", name="xk")
            xq_sb = xin.tile([P, KD, S], BF16, tag="x", name="xq")
            xv_sb = xin.tile([P, KD, S], BF16, tag="x", name="xv")

            nc.sync.dma_start(wk_sb[:], wk.rearrange("(k p) e -> p k e", p=P))
            nc.sync.dma_start(wq_sb[:], wq.rearrange("(k p) e -> p k e", p=P))
            nc.sync.dma_start(bq_sb[:], bq.rearrange("(m p) -> p m", p=P))
            for k in range(KD):     # xq n-block 0 on the sync ring
                nc.sync.dma_start(xq_sb[:, k, 0:NB], xqT[k * P:(k + 1) * P, 0:NB])
            nc.sync.dma_start(wv_sb[:], wv.rearrange("(k p) e -> p k e", p=P))
            nc.sync.dma_start(wo_sb[:], wo.rearrange("(m p) d -> p m d", p=P))

            H = S // 2
            for k in range(KD):     # xk half 0 (cols 0:1024)
                nc.gpsimd.dma_start(xk_sb[:, k, 0:H], xkT[k * P:(k + 1) * P, 0:H])
            for k in range(KD):     # xv half 0
                nc.gpsimd.dma_start(xv_sb[:, k, 0:H], xvT[k * P:(k + 1) * P, 0:H])
            for k in range(KD):     # xk half 1
                nc.gpsimd.dma_start(xk_sb[:, k, H:S], xkT[k * P:(k + 1) * P, H:S])
            for k in range(KD):     # xv half 1
                nc.gpsimd.dma_start(xv_sb[:, k, H:S], xvT[k * P:(k + 1) * P, H:S])
            for k in range(KD):     # xq n-blocks 1-3
                nc.gpsimd.dma_start(xq_sb[:, k, NB:S], xqT[k * P:(k + 1) * P, NB:S])

            # PE warm-up: ~4us of dummy matmuls while the DMAs stream
            wu = psU.tile([P, NB], F32, tag="u", name="warmup")
            for i in range(9):
                nc.tensor.matmul(wu[:], lhsT=dum_sb[:, 0:P], rhs=dum_sb[:],
                                 start=(i == 0), stop=(i == 8))
            nc.vector.tensor_copy(dum_sb[0:1, 0:4], wu[0:1, 0:4])

            proj_ps = {}

            def qk_proj_half(x_sb, w_sb, b_sb, dst, m, n, half):
                """3 of the 6 contraction matmuls of a projection block;
                half==1 finishes and evacuates.  Split so fillers stay
                ~0.7us and never pile up in the PE stream."""
                key = (dst.name, m, n)
                if half == 0:
                    proj_ps[key] = psU.tile([P, NB], F32, tag="u",
                                            name=f"pj{dst.name}_{m}_{n}")
                ps = proj_ps[key]
                for k in range(3 * half, 3 * half + 3):
                    nc.tensor.matmul(
                        ps[:],
                        lhsT=w_sb[:, k, m * P:(m + 1) * P],
                        rhs=x_sb[:, k, n * NB:(n + 1) * NB],
                        start=(k == 0),
                        stop=(k == KD - 1),
                    )
                if half == 0:
                    return
                del proj_ps[key]
                if b_sb is None:
                    nc.vector.tensor_copy(dst[:, m, n * NB:(n + 1) * NB], ps[:])
                else:
                    nc.vector.tensor_scalar(
                        out=dst[:, m, n * NB:(n + 1) * NB],
                        in0=ps[:],
                        scalar1=b_sb[:, m:m + 1],
                        scalar2=None,
                        op0=add,
                    )

            def qk_proj_block(x_sb, w_sb, b_sb, dst, m, n):
                qk_proj_half(x_sb, w_sb, b_sb, dst, m, n, 0)
                qk_proj_half(x_sb, w_sb, b_sb, dst, m, n, 1)

            def v_proj_chunk(s):
                ps = psU.tile([P, GE], F32, tag="u", name=f"pv{s}")
                for k in range(KD):
                    nc.tensor.matmul(
                        ps[:],
                        lhsT=xv_sb[:, k, s * P:(s + 1) * P],
                        rhs=wv_sb[:, k, :],
                        start=(k == 0),
                        stop=(k == KD - 1),
                    )
                ps_h = ps.rearrange("p (h d) -> p h d", d=HEAD_DIM)
                # even local heads -> cols [0:64], odd -> cols [64:128]
                nc.vector.tensor_copy(
                    v_sb[:, s, 0::2, 0:HEAD_DIM], ps_h[:, 0::2, :]
                )
                nc.vector.tensor_copy(
                    v_sb[:, s, 1::2, HEAD_DIM:P], ps_h[:, 1::2, :]
                )

            # out-projection in half-s-chunk units (2 matmuls + 1 copy)
            def out_proj_unit(s, half):
                lo, hi = (0, NB) if half == 0 else (NB, DIM)
                py = psU.tile([P, NB], F32, tag="u", name=f"py{s}_{half}")
                for k in range(MQ):
                    nc.tensor.matmul(
                        py[:, 0:hi - lo],
                        lhsT=ot_sb[:, k, s * P:(s + 1) * P],
                        rhs=wo_sb[:, k, lo:hi],
                        start=(k == 0),
                        stop=(k == MQ - 1),
                    )
                if half == 0:
                    out_proj_unit.y[s] = yout.tile([P, DIM], F32, tag="y",
                                                   name=f"y{s}")
                y_sb = out_proj_unit.y[s]
                nc.vector.tensor_copy(y_sb[:, lo:hi], py[:, 0:hi - lo])
                if half == 1:
                    nc.sync.dma_start(out[s * P:(s + 1) * P, :], y_sb[:])
            out_proj_unit.y = {}
            out_proj_unit.todo = 0
            out_proj_unit.avail = 0

            def drain_out_proj():
                if out_proj_unit.todo < out_proj_unit.avail:
                    unit = out_proj_unit.todo
                    out_proj_unit(unit // 2, unit % 2)
                    out_proj_unit.todo = unit + 1

            def make_normalize(pu, hp, q):
                """1/rowsum via 2-step Newton from a constant seed (~1e-6).
                Head j=0's chain on VectorE, j=1's on GpSimd (parallel);
                only the 64 replicated R rows are copied out — the final
                multiply reads U straight from PSUM."""
                def _norm():
                    for j in range(2):
                        eng = nc.vector if j == 0 else nc.gpsimd
                        dma = nc.sync if j == 0 else nc.gpsimd
                        ulo, uhi = j * HEAD_DIM, (j + 1) * HEAD_DIM
                        rlo, rhi = (1 - j) * HEAD_DIM, (2 - j) * HEAD_DIM
                        ur = nrm.tile([P, NB], F32, tag=f"ur{j}",
                                      name=f"ur{hp}_{q}_{j}")
                        nc.vector.tensor_copy(ur[rlo:rhi, :], pu[j][rlo:rhi, :])
                        rr = ur[rlo:rhi, :]
                        x1 = nrm.tile([P, NB], F32, tag=f"x1{j}")
                        tmp = nrm.tile([P, NB], F32, tag=f"tmp{j}")
                        eng.tensor_scalar(       # x1 = 2x0 - x0^2 r
                            out=x1[rlo:rhi, :], in0=rr,
                            scalar1=-X0 * X0, scalar2=2.0 * X0,
                            op0=mult, op1=add,
                        )
                        eng.tensor_tensor(       # e = r * x1
                            out=tmp[rlo:rhi, :], in0=rr,
                            in1=x1[rlo:rhi, :], op=mult,
                        )
                        eng.tensor_scalar(       # u = 2 - e
                            out=tmp[rlo:rhi, :], in0=tmp[rlo:rhi, :],
                            scalar1=-1.0, scalar2=2.0,
                            op0=mult, op1=add,
                        )
                        eng.tensor_tensor(       # x2 = x1 * u
                            out=x1[rlo:rhi, :], in0=x1[rlo:rhi, :],
                            in1=tmp[rlo:rhi, :], op=mult,
                        )
                        # recip rows onto U partitions, then scale into O^T
                        dma.dma_start(x1[ulo:uhi, :], x1[rlo:rhi, :])
                        nc.vector.tensor_tensor(
                            out=ot_sb[ulo:uhi, hp, q * NB:(q + 1) * NB],
                            in0=pu[j][ulo:uhi, :],
                            in1=x1[ulo:uhi, :],
                            op=mult,
                        )
                return _norm

            # ---------- per-chunk filler schedule ----------
            # sched[gc] -> PE-work closures drained at global chunk gc
            # (gc = 16*(2q+hp) + m; chunk gc runs ~ 8 + 1.1*gc us).  Each
            # closure's data deps are landed (or land within ~1us) by the
            # time the PE reaches it, so the in-order PE stream never
            # head-blocks.
            sched = {}

            def at(gc, fn):
                sched.setdefault(gc, []).append(fn)

            # Remaining projections ride blocks 0-1 as halves (~0.7us
            # fillers): hp0's K-proj n1-n3 first (consumed within block 0),
            # hp1's K-proj + Q0-proj e-chunk 1 (consumed from block 1),
            # V-proj paced 1/chunk behind the xv halves.
            def KPH(m, n, half):
                return lambda: qk_proj_half(xk_sb, wk_sb, None, kt_sb, m, n, half)

            def QPH(m, n, half):
                return lambda: qk_proj_half(xq_sb, wq_sb, bq_sb, qt_sb, m, n, half)
            at(0, KPH(0, 1, 0))
            at(1, KPH(0, 1, 1))
            at(2, KPH(1, 0, 0))
            at(3, KPH(1, 0, 1))
            at(4, QPH(1, 0, 0))
            at(5, QPH(1, 0, 1))
            at(6, KPH(0, 2, 0))
            at(7, KPH(0, 2, 1))
            at(8, KPH(0, 3, 0))
            at(9, KPH(0, 3, 1))
            at(10, KPH(1, 1, 0))
            at(11, KPH(1, 1, 1))
            at(12, KPH(1, 2, 0))
            at(13, KPH(1, 2, 1))
            at(14, KPH(1, 3, 0))
            at(15, KPH(1, 3, 1))
            # all V-proj inside block 0 (its fillers may allocate PSUM
            # slots; from block 1 all four slots hold PV accumulators)
            VP_SLOTS = [9, 10, 10, 11, 11, 12, 12, 13, 13,
                        14, 14, 14, 15, 15, 15, 15]
            for s in range(SC):
                at(VP_SLOTS[s], lambda s=s: v_proj_chunk(s))

            pv_queue = []            # deferred PV chunk closures, FIFO
            qk_partial = {}

            def q_phase(qn, mq_idx, phase):
                if phase == 0:
                    qp = psU.tile([P, NB], F32, tag="u", name=f"qp{qn}_{mq_idx}")
                    qk_partial[mq_idx] = qp
                qp = qk_partial[mq_idx]
                for k in (2 * phase, 2 * phase + 1):
                    nc.tensor.matmul(
                        qp[:],
                        lhsT=wq_sb[:, k, mq_idx * P:(mq_idx + 1) * P],
                        rhs=xq_sb[:, k, qn * NB:(qn + 1) * NB],
                        start=(k == 0),
                        stop=(k == KD - 1),
                    )
                if phase == 2:
                    nc.vector.tensor_scalar(
                        out=qt_sb[:, mq_idx, qn * NB:(qn + 1) * NB],
                        in0=qp[:],
                        scalar1=bq_sb[:, mq_idx:mq_idx + 1],
                        scalar2=None,
                        op0=add,
                    )

            # ---- prelude: only e-chunk 0 of K/Q n-block 0 (all block 0
            # needs to start); e-chunk 1 rides the block-0 fillers ----
            qk_proj_block(xk_sb, wk_sb, None, kt_sb, 0, 0)
            qk_proj_block(xq_sb, wq_sb, bq_sb, qt_sb, 0, 0)

            pend = []

            # ---- attention, one (q, head-pair) block at a time ----
            for q in range(NQ):
                for hp in range(MQ):
                    bi = 2 * q + hp
                    pu = [
                        psU.tile([P, NB], F32, tag="u",
                                 name=f"pu{hp}_{q}_{j}")
                        for j in range(2)
                    ]
                    for m in range(SC):
                        gc = 16 * bi + m
                        # previous block's deferred normalize once this
                        # block is under way and its accumulators are
                        # complete (the PV backlog has fully drained)
                        if m >= 2 and pend and not pv_queue:
                            pend.pop(0)()
                        ss = psS.tile([P, 2, NB], F32, tag="s")
                        for j in range(2):
                            lo, hi = j * HEAD_DIM, (j + 1) * HEAD_DIM
                            nc.tensor.matmul(
                                ss[:, j, :],
                                lhsT=kt_sb[lo:hi, hp, m * P:(m + 1) * P],
                                rhs=qt_sb[lo:hi, hp, q * NB:(q + 1) * NB],
                                start=True,
                                stop=True,
                            )
                        if m < N_DVE:
                            # Schraudolph bit-exp on DVE: bf16 bits via
                            # int16 convert of s*A+B, written into a bf16
                            # tile through a bitcast view
                            es = esp.tile([P, 2, NB], BF16, tag="es",
                                          name=f"es{bi}_{m}")
                            nc.vector.tensor_scalar(
                                out=es[:].bitcast(I16),
                                in0=ss[:],
                                scalar1=SCH_A, scalar2=SCH_B,
                                op0=mult, op1=add,
                            )
                        else:
                            es = esp.tile([P, 2, NB], BF16, tag="es",
                                          name=f"es{bi}_{m}")
                            nc.scalar.activation(es[:], ss[:], Exp, scale=SCALE)

                        def pv(pu=pu, hp=hp, m=m, es=es):
                            for j in range(2):
                                nc.tensor.matmul(
                                    pu[j][:],
                                    lhsT=v_sb[:, m, 2 * hp + j, :],
                                    rhs=es[:, j, :],
                                    start=(m == 0),
                                    stop=(m == SC - 1),
                                )
                        # PV runs inline once the backlog is clear (from
                        # mid-block-1 on); blocks 0/1 defer through the
                        # queue while V-proj catches up with the xv DMAs
                        if pv_queue or bi == 0:
                            pv_queue.append(pv)
                        else:
                            pv()
                        # scheduled fillers (K/V projections) BEFORE the
                        # queue drains that consume their outputs
                        for fn in sched.pop(gc, ()):
                            fn()
                        # drain the PV backlog: 1/chunk inside block 0
                        # (V-proj is still catching up), 3/chunk in block
                        # 1 so the queue is dry before q_phase needs PSUM
                        if pv_queue and gc >= PV_DRAIN_FROM:
                            budget = 1 if bi == 0 else 3
                            for _ in range(budget):
                                if pv_queue:
                                    pv_queue.pop(0)()
                        # out-proj half-units: 4 late in hp0 blocks (after
                        # the previous normalize's ~5us latency), 4 early
                        # in hp1 blocks
                        if (hp == 0 and m in (8, 10, 12, 14)) or \
                           (hp == 1 and m in (2, 4, 6, 8)):
                            drain_out_proj()
                        # next q block's Q^T projection, 2 matmuls a time,
                        # finishing 3 chunks before the next block uses
                        # qt (block 1 holds no free PSUM slot before its
                        # backlog normalize at ~m10)
                        q_slots = (10, 11, 12, 13, 14, 15) if bi == 1 else (7, 8, 9, 10, 11, 12)
                        if hp == 1 and q + 1 < NQ and m in q_slots:
                            ph = m - q_slots[0]
                            mq_idx, phase = divmod(ph, 3)
                            q_phase(q + 1, mq_idx, phase)
                    # safety: by design the queue is empty from block 2 on
                    if bi >= 2:
                        while pv_queue:
                            pv_queue.pop(0)()
                    last = (q == NQ - 1 and hp == MQ - 1)
                    nrm_fn = make_normalize(pu, hp, q)
                    if last:
                        nrm_fn()
                    else:
                        pend.append(nrm_fn)
                    if hp == 1:
                        out_proj_unit.avail = 8 * (q + 1)
            for th in pend:
                th()
            # ---- tail: the last q-block's out-projection; evacuations
            # alternate VectorE/ScalarE (ScalarE is idle by now) so the
            # PSUM drain doesn't serialize on one engine ----
            for unit in range(out_proj_unit.todo, 8 * NQ):
                s, half = unit // 2, unit % 2
                lo, hi = (0, NB) if half == 0 else (NB, DIM)
                py = psU.tile([P, NB], F32, tag="u", name=f"tpy{s}_{half}")
                for k in range(MQ):
                    nc.tensor.matmul(
                        py[:, 0:hi - lo],
                        lhsT=ot_sb[:, k, s * P:(s + 1) * P],
                        rhs=wo_sb[:, k, lo:hi],
                        start=(k == 0),
                        stop=(k == MQ - 1),
                    )
                if half == 0:
                    out_proj_unit.y[s] = yout.tile([P, DIM], F32, tag="y",
                                                   name=f"y{s}")
                y_sb = out_proj_unit.y[s]
                if half == 0:
                    nc.vector.tensor_copy(y_sb[:, lo:hi], py[:, 0:hi - lo])
                else:
                    nc.scalar.copy(y_sb[:, lo:hi], py[:, 0:hi - lo])
                    nc.sync.dma_start(out[s * P:(s + 1) * P, :], y_sb[:])

    if split_waits:
        _split_multi_waits(nc)
    return nc


_NC = None


def _get_nc():
    global _NC
    if _NC is None:
        _NC = build_nc()
    return _NC


def _bf(a):
    return np.ascontiguousarray(np.asarray(a, dtype=np.float32)).astype(NPBF16)


def make_in_maps(query, key, value, wq, bq, wk, bk, wv, bv, wo, bo):
    query = np.asarray(query, np.float32)
    key = np.asarray(key, np.float32)
    value = np.asarray(value, np.float32)
    wq = np.asarray(wq, np.float32)
    wk = np.asarray(wk, np.float32)
    wv = np.asarray(wv, np.float32)
    wo = np.asarray(wo, np.float32)
    in_maps = []
    for b in range(B):
        xqT = _bf(query[b].T)
        xkT = _bf(key[b].T)
        xvT = _bf(value[b].T)
        for g in range(GROUPS):
            sl = slice(g * GE, (g + 1) * GE)
            in_maps.append({
                "xqT": xqT,
                "xkT": xkT,
                "xvT": xvT,
                "wq": _bf(wq[:, sl]),
                "wk": _bf(wk[:, sl]),
                "wv": _bf(wv[:, sl]),
                "wo": _bf(wo[sl, :]),
                "bq": np.ascontiguousarray(np.asarray(bq, np.float32)[sl]),
            })
    return in_maps


def kernel(query, key, value, wq, bq, wk, bk, wv, bv, wo, bo, **kw):
    nc = _get_nc()
    in_maps = make_in_maps(query, key, value, wq, bq, wk, bk, wv, bv, wo, bo)
    res = run_bass_kernel_spmd(nc, in_maps, list(range(NCORES))).results
    # bv is dropped on device (softmax rows sum to 1) and folded here;
    # bk shifts scores by a per-query constant and is softmax-invariant.
    fold = (np.asarray(bv, np.float32) @ np.asarray(wo, np.float32)
            + np.asarray(bo, np.float32))
    out = np.empty((B, S, DIM), np.float32)
    for b in range(B):
        out[b] = res[b * GROUPS]["out"] + res[b * GROUPS + 1]["out"] + fold
    return out


# revision 31
# speedup vs baseline: 1.0550x; 1.0550x over previous
"""Trainium2 Bass kernel for nn_MultiHeadAttention (B=4, S=2048, DIM=768,
EMBED=512, HEADS=8, HEAD_DIM=64), distributed over 8 NeuronCores.

Sharding: core (b, g) with b in 0..3 (batch, data parallel) and g in 0..1
(head-group of 4 heads, tensor parallel). Each core computes a partial
output Y_partial[b,g] = softmax(QK^T/8) V @ Wo[g-slice]; the host sums the
two group partials per batch and adds the fused output bias bv@wo + bo
(bv is removable on device because softmax rows sum to 1; bk only shifts
each softmax row by a per-query constant, so it is dropped entirely).

Device dataflow per core (bf16 matmuls, fp32 PSUM accumulation):
  - host supplies x^T (DIM on partitions) so no on-device transposes exist
  - Q^T = Wq^T x^T + bq, K^T = Wk^T x^T (no bias) -> [256, S]
  - V packed as [V_h | ones] / [ones | V_h] per local head so the PV
    matmul also emits the softmax denominator replicated 64x
  - S^T via lhsT=K^T chunk, rhs=Q^T block; two heads run as concurrent
    K=64 PE tiles (row packing)
  - exp on ScalarE ACTIVATE(Exp, scale=1/8) straight out of PSUM, FD=1024;
    the first N_DVE chunks of each block run on VectorE instead via a
    Schraudolph bit-exp (int16(s*A+B) bitcast to bf16, ~2-4% sawtooth
    that mostly cancels in the softmax ratio)
  - 1/rowsum via 2-step Newton from a constant seed on GpSimd (all-SBUF),
    keeping VectorE free for PSUM evacuations
  - O^T.T @ Wo -> [S, DIM], evac + DMA per s-chunk
Scheduling: activations stream in [128,512] pieces split across both DMA
rings in consumption order (xk-n0, xq-n0, xk-n1, xv-n0, xv-n1, xk-n2/3,
xv-n2/3, xq-n1..3) so the first exp fires ~8us in; K/V projections and the
block-0 PV backlog (V lands late) drain through a per-chunk filler schedule
inside the attention loops so the PE stream never head-blocks on a DMA.
A post-pass splits multi-semaphore waits and the gpsimd RANGE_CLEAR into
single-wait NoOps for this image's stricter walrus.
"""

import numpy as np
import ml_dtypes

import concourse.bass as bass
import concourse.tile as tile
from concourse import mybir
from concourse.bass_utils import run_bass_kernel_spmd

BF16 = mybir.dt.bfloat16
F32 = mybir.dt.float32
I16 = mybir.dt.int16
NPBF16 = ml_dtypes.bfloat16

B, S, DIM, EMBED, HEADS, HEAD_DIM = 4, 2048, 768, 512, 8, 64
P = 128
KD = DIM // P          # 6   contraction chunks for projections
GROUPS = 2             # head-groups (tensor-parallel split)
GE = EMBED // GROUPS   # 256 embed columns per group
GH = HEADS // GROUPS   # 4   heads per group
MQ = GE // P           # 2   e-chunks per group
SC = S // P            # 16  sequence chunks of 128
NB = 512               # matmul free-dim block
NQ = S // NB           # 4   query blocks
SCALE = 0.125          # 1/sqrt(HEAD_DIM)
NCORES = B * GROUPS    # 8
X0 = 1.0 / 2146.0      # Newton seed for 1/rowsum

# Schraudolph bit-exp on DVE for the first N_DVE chunks of each block:
# bf16 bits of exp(s*SCALE) ~ round(s * 128*log2(e)*SCALE + 127*128 - sig)
N_DVE = 2
SCH_A = 128.0 * 1.4426950408889634 * SCALE
SCH_B = 127.0 * 128.0 - 0.5 * 128.0 * 0.043

ESP_BUFS = 16          # es backlog depth (chunks) for the deferred-PV ramp
PV_DRAIN_FROM = 12     # global chunk index where block-0 PV drains start


def _split_multi_waits(nc):
    """The walrus build in this image accepts at most ONE sem-wait per
    instruction (setupSyncWait: 'Too many sync wait commands'), while Tile
    freely attaches several.  Hoist all but the last wait of each
    instruction onto same-engine NoOps inserted immediately before it —
    identical blocking semantics, one wait per instruction."""
    ctr = 0
    for f in nc.m.functions:
        for blk in f.blocks:
            il = blk.instructions
            out = []
            for inst in il:
                if type(inst).__name__ == "InstISA":
                    # kernel-tail gpsimd.sem_clear (RANGE_CLEAR): this
                    # walrus rejects its encoding ("ISA wrong length").
                    # NRT re-initializes semaphore state per execution, so
                    # replace it with a NoOp carrying the same syncs.
                    nop = mybir.InstNoOp(
                        name=f"{inst.name}-isanop", ins=[], outs=[]
                    )
                    nop.engine = inst.engine
                    nop.sync_info = inst.sync_info
                    out.append(nop)
                    continue
                si = inst.sync_info
                if si is not None and si.on_wait and len(si.on_wait) > 1:
                    waits = list(si.on_wait)
                    for w in waits[:-1]:
                        ctr += 1
                        nop = mybir.InstNoOp(
                            name=f"I-waitsplit-{ctr}", ins=[], outs=[]
                        )
                        nop.engine = inst.engine
                        nop.sync_info = mybir.SyncInfo(on_wait=[w], on_update=[])
                        out.append(nop)
                    si.on_wait = [waits[-1]]
                out.append(inst)
            il[:] = out
    return ctr


def build_nc(split_waits=True):
    nc = bass.Bass("TRN2", target_bir_lowering=False, debug=False)

    xqT = nc.dram_tensor("xqT", [DIM, S], BF16, kind="ExternalInput").ap()
    xkT = nc.dram_tensor("xkT", [DIM, S], BF16, kind="ExternalInput").ap()
    xvT = nc.dram_tensor("xvT", [DIM, S], BF16, kind="ExternalInput").ap()
    wq = nc.dram_tensor("wq", [DIM, GE], BF16, kind="ExternalInput").ap()
    wk = nc.dram_tensor("wk", [DIM, GE], BF16, kind="ExternalInput").ap()
    wv = nc.dram_tensor("wv", [DIM, GE], BF16, kind="ExternalInput").ap()
    wo = nc.dram_tensor("wo", [GE, DIM], BF16, kind="ExternalInput").ap()
    bq = nc.dram_tensor("bq", [GE], F32, kind="ExternalInput").ap()
    out = nc.dram_tensor("out", [S, DIM], F32, kind="ExternalOutput").ap()

    add = mybir.AluOpType.add
    mult = mybir.AluOpType.mult
    Exp = mybir.ActivationFunctionType.Exp

    with tile.TileContext(nc) as tc:
        with (
            tc.tile_pool(name="const", bufs=1) as const,
            # PSUM: "s" = 2 slots x [P,2,NB] (score pairs, 4 banks);
            #       "u" = 4 slots x 1 bank (proj blocks, PV accumulators,
            #             next-q proj, out-proj halves) = 8 banks total.
            tc.tile_pool(name="psS", bufs=2, space="PSUM") as psS,
            tc.tile_pool(name="psU", bufs=4, space="PSUM") as psU,
            tc.tile_pool(name="esp", bufs=ESP_BUFS) as esp,
            tc.tile_pool(name="nrm", bufs=2) as nrm,
            tc.tile_pool(name="yout", bufs=3) as yout,
            tc.tile_pool(name="xin", bufs=3) as xin,
        ):
            wq_sb = const.tile([P, KD, GE], BF16, tag="wq")
            wk_sb = const.tile([P, KD, GE], BF16, tag="wk")
            wv_sb = const.tile([P, KD, GE], BF16, tag="wv")
            wo_sb = const.tile([P, MQ, DIM], BF16, tag="wo")
            bq_sb = const.tile([P, MQ], F32, tag="bq")
            qt_sb = const.tile([P, MQ, S], BF16, tag="qt")   # Q^T
            kt_sb = const.tile([P, MQ, S], BF16, tag="kt")   # K^T
            ot_sb = const.tile([P, MQ, S], BF16, tag="ot")   # O^T
            # V in PV-lhsT layout: per (s-chunk, head) a [128, 128] block
            # of [V_h | ones] (even local head) or [ones | V_h] (odd); the
            # ones columns make the PV matmul also produce the softmax
            # denominator (replicated 64x), partition-aligned with the
            # other head's slot.
            v_sb = const.tile([P, SC, GH, P], BF16, tag="v")
            nc.vector.memset(v_sb[:], 1.0)

            # small ones tile: PE warm-up operand (HAM un-throttles after
            # ~3.4us of sustained matmul activity; warming on dummies while
            # the first DMAs stream means the real prelude runs at 2.4GHz).
            # memset on gpsimd: the DVE is busy with the big v_sb memset.
            dum_sb = const.tile([P, NB], BF16, tag="dum")
            nc.gpsimd.memset(dum_sb[:], 1.0)

            # --- DMAs: sync ring carries weights + xq n-block 0 (so the
            # Q0 projection starts early); gpsimd ring carries the rest in
            # [128,1024] halves (DMA triggers cost ~0.64us each, so pieces
            # stay coarse), ordered by first consumption.
            xk_sb = xin.tile([P, KD, S], BF16, tag="x", name="xk")
            xq_sb = xin.tile([P, KD, S], BF16, tag="x", name="xq")
            xv_sb = xin.tile([P, KD, S], BF16, tag="x", name="xv")

            nc.sync.dma_start(wk_sb[:], wk.rearrange("(k p) e -> p k e", p=P))
            nc.sync.dma_start(wq_sb[:], wq.rearrange("(k p) e -> p k e", p=P))
            nc.sync.dma_start(bq_sb[:], bq.rearrange("(m p) -> p m", p=P))
            for k in range(KD):     # xq n-block 0 on the sync ring
                nc.sync.dma_start(xq_sb[:, k, 0:NB], xqT[k * P:(k + 1) * P, 0:NB])
            nc.sync.dma_start(wv_sb[:], wv.rearrange("(k p) e -> p k e", p=P))
            nc.sync.dma_start(wo_sb[:], wo.rearrange("(m p) d -> p m d", p=P))

            H = S // 2
            for k in range(KD):     # xk half 0 (cols 0:1024)
                nc.gpsimd.dma_start(xk_sb[:, k, 0:H], xkT[k * P:(k + 1) * P, 0:H])
            for k in range(KD):     # xv half 0
                nc.gpsimd.dma_start(xv_sb[:, k, 0:H], xvT[k * P:(k + 1) * P, 0:H])
            for k in range(KD):     # xk half 1
                nc.gpsimd.dma_start(xk_sb[:, k, H:S], xkT[k * P:(k + 1) * P, H:S])
            for k in range(KD):     # xv half 1
                nc.gpsimd.dma_start(xv_sb[:, k, H:S], xvT[k * P:(k + 1) * P, H:S])
            for k in range(KD):     # xq n-blocks 1-3
                nc.gpsimd.dma_start(xq_sb[:, k, NB:S], xqT[k * P:(k + 1) * P, NB:S])

            # PE warm-up: ~4us of dummy matmuls while the DMAs stream
            wu = psU.tile([P, NB], F32, tag="u", name="warmup")
            for i in range(9):
                nc.tensor.matmul(wu[:], lhsT=dum_sb[:, 0:P], rhs=dum_sb[:],
                                 start=(i == 0), stop=(i == 8))
            nc.vector.tensor_copy(dum_sb[0:1, 0:4], wu[0:1, 0:4])

            proj_ps = {}

            def qk_proj_half(x_sb, w_sb, b_sb, dst, m, n, half):
                """3 of the 6 contraction matmuls of a projection block;
                half==1 finishes and evacuates.  Split so fillers stay
                ~0.7us and never pile up in the PE stream."""
                key = (dst.name, m, n)
                if half == 0:
                    proj_ps[key] = psU.tile([P, NB], F32, tag="u",
                                            name=f"pj{dst.name}_{m}_{n}")
                ps = proj_ps[key]
                for k in range(3 * half, 3 * half + 3):
                    nc.tensor.matmul(
                        ps[:],
                        lhsT=w_sb[:, k, m * P:(m + 1) * P],
                        rhs=x_sb[:, k, n * NB:(n + 1) * NB],
                        start=(k == 0),
                        stop=(k == KD - 1),
                    )
                if half == 0:
                    return
                del proj_ps[key]
                if b_sb is None:
                    nc.vector.tensor_copy(dst[:, m, n * NB:(n + 1) * NB], ps[:])
                else:
                    nc.vector.tensor_scalar(
                        out=dst[:, m, n * NB:(n + 1) * NB],
                        in0=ps[:],
                        scalar1=b_sb[:, m:m + 1],
                        scalar2=None,
                        op0=add,
                    )

            def qk_proj_block(x_sb, w_sb, b_sb, dst, m, n):
                qk_proj_half(x_sb, w_sb, b_sb, dst, m, n, 0)
                qk_proj_half(x_sb, w_sb, b_sb, dst, m, n, 1)

            def v_proj_chunk(s):
                ps = psU.tile([P, GE], F32, tag="u", name=f"pv{s}")
                for k in range(KD):
                    nc.tensor.matmul(
                        ps[:],
                        lhsT=xv_sb[:, k, s * P:(s + 1) * P],
                        rhs=wv_sb[:, k, :],
                        start=(k == 0),
                        stop=(k == KD - 1),
                    )
                ps_h = ps.rearrange("p (h d) -> p h d", d=HEAD_DIM)
                # even local heads -> cols [0:64], odd -> cols [64:128]
                nc.vector.tensor_copy(
                    v_sb[:, s, 0::2, 0:HEAD_DIM], ps_h[:, 0::2, :]
                )
                nc.vector.tensor_copy(
                    v_sb[:, s, 1::2, HEAD_DIM:P], ps_h[:, 1::2, :]
                )

            # out-projection in half-s-chunk units (2 matmuls + 1 copy)
            def out_proj_unit(s, half):
                lo, hi = (0, NB) if half == 0 else (NB, DIM)
                py = psU.tile([P, NB], F32, tag="u", name=f"py{s}_{half}")
                for k in range(MQ):
                    nc.tensor.matmul(
                        py[:, 0:hi - lo],
                        lhsT=ot_sb[:, k, s * P:(s + 1) * P],
                        rhs=wo_sb[:, k, lo:hi],
                        start=(k == 0),
                        stop=(k == MQ - 1),
                    )
                if half == 0:
                    out_proj_unit.y[s] = yout.tile([P, DIM], F32, tag="y",
                                                   name=f"y{s}")
                y_sb = out_proj_unit.y[s]
                nc.vector.tensor_copy(y_sb[:, lo:hi], py[:, 0:hi - lo])
                if half == 1:
                    nc.sync.dma_start(out[s * P:(s + 1) * P, :], y_sb[:])
            out_proj_unit.y = {}
            out_proj_unit.todo = 0
            out_proj_unit.avail = 0

            def drain_out_proj():
                if out_proj_unit.todo < out_proj_unit.avail:
                    unit = out_proj_unit.todo
                    out_proj_unit(unit // 2, unit % 2)
                    out_proj_unit.todo = unit + 1

            def make_normalize(pu, hp, q):
                """1/rowsum via 2-step Newton from a constant seed (~1e-6).
                Head j=0's chain on VectorE, j=1's on GpSimd, emitted
                interleaved so both run concurrently and the recip-DMA
                waits never head-block the ur copies."""
                def _norm():
                    ur, x1, tmp, eng, rsl, usl = {}, {}, {}, {}, {}, {}
                    for j in range(2):
                        eng[j] = nc.vector if j == 0 else nc.gpsimd
                        usl[j] = slice(j * HEAD_DIM, (j + 1) * HEAD_DIM)
                        rsl[j] = slice((1 - j) * HEAD_DIM, (2 - j) * HEAD_DIM)
                        ur[j] = nrm.tile([P, NB], F32, tag=f"ur{j}",
                                         name=f"ur{hp}_{q}_{j}")
                        # full copy: releases the PV bank early
                        nc.vector.tensor_copy(ur[j][:], pu[j][:])
                        x1[j] = nrm.tile([P, NB], F32, tag=f"x1{j}",
                                         name=f"x1{hp}_{q}_{j}")
                        tmp[j] = nrm.tile([P, NB], F32, tag=f"tmp{j}",
                                          name=f"tmp{hp}_{q}_{j}")
                    for j in range(2):
                        eng[j].tensor_scalar(    # x1 = 2x0 - x0^2 r
                            out=x1[j][rsl[j], :], in0=ur[j][rsl[j], :],
                            scalar1=-X0 * X0, scalar2=2.0 * X0,
                            op0=mult, op1=add,
                        )
                    for j in range(2):
                        eng[j].tensor_tensor(    # e = r * x1
                            out=tmp[j][rsl[j], :], in0=ur[j][rsl[j], :],
                            in1=x1[j][rsl[j], :], op=mult,
                        )
                        eng[j].tensor_scalar(    # u = 2 - e
                            out=tmp[j][rsl[j], :], in0=tmp[j][rsl[j], :],
                            scalar1=-1.0, scalar2=2.0,
                            op0=mult, op1=add,
                        )
                        eng[j].tensor_tensor(    # x2 = x1 * u
                            out=x1[j][rsl[j], :], in0=x1[j][rsl[j], :],
                            in1=tmp[j][rsl[j], :], op=mult,
                        )
                    # recip rows onto U partitions, then scale into O^T
                    nc.sync.dma_start(x1[0][usl[0], :], x1[0][rsl[0], :])
                    nc.gpsimd.dma_start(x1[1][usl[1], :], x1[1][rsl[1], :])
                    for j in range(2):
                        nc.vector.tensor_tensor(
                            out=ot_sb[usl[j], hp, q * NB:(q + 1) * NB],
                            in0=ur[j][usl[j], :],
                            in1=x1[j][usl[j], :],
                            op=mult,
                        )
                return _norm

            # ---------- per-chunk filler schedule ----------
            # sched[gc] -> PE-work closures drained at global chunk gc
            # (gc = 16*(2q+hp) + m; chunk gc runs ~ 8 + 1.1*gc us).  Each
            # closure's data deps are landed (or land within ~1us) by the
            # time the PE reaches it, so the in-order PE stream never
            # head-blocks.
            sched = {}

            def at(gc, fn):
                sched.setdefault(gc, []).append(fn)

            # Remaining projections ride blocks 0-1 as halves (~0.7us
            # fillers): hp0's K-proj n1-n3 first (consumed within block 0),
            # hp1's K-proj + Q0-proj e-chunk 1 (consumed from block 1),
            # V-proj paced 1/chunk behind the xv halves.
            def KPH(m, n, half):
                return lambda: qk_proj_half(xk_sb, wk_sb, None, kt_sb, m, n, half)

            def QPH(m, n, half):
                return lambda: qk_proj_half(xq_sb, wq_sb, bq_sb, qt_sb, m, n, half)
            at(0, KPH(0, 1, 0))
            at(1, KPH(0, 1, 1))
            at(2, KPH(1, 0, 0))
            at(3, KPH(1, 0, 1))
            at(4, QPH(1, 0, 0))
            at(5, QPH(1, 0, 1))
            at(6, KPH(0, 2, 0))
            at(7, KPH(0, 2, 1))
            at(8, KPH(0, 3, 0))
            at(9, KPH(0, 3, 1))
            at(10, KPH(1, 1, 0))
            at(11, KPH(1, 1, 1))
            at(12, KPH(1, 2, 0))
            at(13, KPH(1, 2, 1))
            at(14, KPH(1, 3, 0))
            at(15, KPH(1, 3, 1))
            # all V-proj inside block 0 (its fillers may allocate PSUM
            # slots; from block 1 all four slots hold PV accumulators)
            VP_SLOTS = [9, 10, 10, 11, 11, 12, 12, 13, 13,
                        14, 14, 14, 15, 15, 15, 15]
            for s in range(SC):
                at(VP_SLOTS[s], lambda s=s: v_proj_chunk(s))

            pv_queue = []            # deferred PV chunk closures, FIFO
            qk_partial = {}

            def q_phase(qn, mq_idx, phase):
                if phase == 0:
                    qp = psU.tile([P, NB], F32, tag="u", name=f"qp{qn}_{mq_idx}")
                    qk_partial[mq_idx] = qp
                qp = qk_partial[mq_idx]
                for k in (2 * phase, 2 * phase + 1):
                    nc.tensor.matmul(
                        qp[:],
                        lhsT=wq_sb[:, k, mq_idx * P:(mq_idx + 1) * P],
                        rhs=xq_sb[:, k, qn * NB:(qn + 1) * NB],
                        start=(k == 0),
                        stop=(k == KD - 1),
                    )
                if phase == 2:
                    nc.vector.tensor_scalar(
                        out=qt_sb[:, mq_idx, qn * NB:(qn + 1) * NB],
                        in0=qp[:],
                        scalar1=bq_sb[:, mq_idx:mq_idx + 1],
                        scalar2=None,
                        op0=add,
                    )

            # ---- prelude: only e-chunk 0 of K/Q n-block 0 (all block 0
            # needs to start); e-chunk 1 rides the block-0 fillers ----
            qk_proj_block(xk_sb, wk_sb, None, kt_sb, 0, 0)
            qk_proj_block(xq_sb, wq_sb, bq_sb, qt_sb, 0, 0)

            pend = []

            # ---- attention, one (q, head-pair) block at a time ----
            for q in range(NQ):
                for hp in range(MQ):
                    bi = 2 * q + hp
                    pu = [
                        psU.tile([P, NB], F32, tag="u",
                                 name=f"pu{hp}_{q}_{j}")
                        for j in range(2)
                    ]
                    for m in range(SC):
                        gc = 16 * bi + m
                        # previous block's deferred normalize once this
                        # block is under way and its accumulators are
                        # complete (the PV backlog has fully drained)
                        if m >= 2 and pend and not pv_queue:
                            pend.pop(0)()
                        ss = psS.tile([P, 2, NB], F32, tag="s")
                        for j in range(2):
                            lo, hi = j * HEAD_DIM, (j + 1) * HEAD_DIM
                            nc.tensor.matmul(
                                ss[:, j, :],
                                lhsT=kt_sb[lo:hi, hp, m * P:(m + 1) * P],
                                rhs=qt_sb[lo:hi, hp, q * NB:(q + 1) * NB],
                                start=True,
                                stop=True,
                            )
                        if m < N_DVE:
                            # Schraudolph bit-exp on DVE: bf16 bits via
                            # int16 convert of s*A+B, written into a bf16
                            # tile through a bitcast view
                            es = esp.tile([P, 2, NB], BF16, tag="es",
                                          name=f"es{bi}_{m}")
                            nc.vector.tensor_scalar(
                                out=es[:].bitcast(I16),
                                in0=ss[:],
                                scalar1=SCH_A, scalar2=SCH_B,
                                op0=mult, op1=add,
                            )
                        else:
                            es = esp.tile([P, 2, NB], BF16, tag="es",
                                          name=f"es{bi}_{m}")
                            nc.scalar.activation(es[:], ss[:], Exp, scale=SCALE)

                        def pv(pu=pu, hp=hp, m=m, es=es):
                            for j in range(2):
                                nc.tensor.matmul(
                                    pu[j][:],
                                    lhsT=v_sb[:, m, 2 * hp + j, :],
                                    rhs=es[:, j, :],
                                    start=(m == 0),
                                    stop=(m == SC - 1),
                                )
                        # PV runs inline once the backlog is clear (from
                        # mid-block-1 on); blocks 0/1 defer through the
                        # queue while V-proj catches up with the xv DMAs
                        if pv_queue or bi == 0:
                            pv_queue.append(pv)
                        else:
                            pv()
                        # scheduled fillers (K/V projections) BEFORE the
                        # queue drains that consume their outputs
                        for fn in sched.pop(gc, ()):
                            fn()
                        # drain the PV backlog: 1/chunk inside block 0
                        # (V-proj is still catching up), 3/chunk in block
                        # 1 so the queue is dry before q_phase needs PSUM
                        if pv_queue and gc >= PV_DRAIN_FROM:
                            budget = 1 if bi == 0 else 3
                            for _ in range(budget):
                                if pv_queue:
                                    pv_queue.pop(0)()
                        # out-proj half-units: 3 late in hp0 blocks (the
                        # previous normalize's ~7us chain gates ot), 5
                        # spread through hp1 blocks
                        if (hp == 0 and m in (10, 12, 14)) or \
                           (hp == 1 and m in (0, 2, 4, 6, 8)):
                            drain_out_proj()
                        # next q block's Q^T projection, 2 matmuls a time,
                        # finishing 3 chunks before the next block uses
                        # qt (block 1 holds no free PSUM slot before its
                        # backlog normalize at ~m10)
                        q_slots = (10, 11, 12, 13, 14, 15) if bi == 1 else (8, 9, 10, 11, 12, 13)
                        if hp == 1 and q + 1 < NQ and m in q_slots:
                            ph = m - q_slots[0]
                            mq_idx, phase = divmod(ph, 3)
                            q_phase(q + 1, mq_idx, phase)
                    # safety: by design the queue is empty from block 2 on
                    if bi >= 2:
                        while pv_queue:
                            pv_queue.pop(0)()
                    last = (q == NQ - 1 and hp == MQ - 1)
                    nrm_fn = make_normalize(pu, hp, q)
                    if last:
                        nrm_fn()
                    else:
                        pend.append(nrm_fn)
                    if hp == 1:
                        out_proj_unit.avail = 8 * (q + 1)
            for th in pend:
                th()
            # ---- tail: the last q-block's out-projection; evacuations
            # alternate VectorE/ScalarE (ScalarE is idle by now) so the
            # PSUM drain doesn't serialize on one engine ----
            for unit in range(out_proj_unit.todo, 8 * NQ):
                s, half = unit // 2, unit % 2
                lo, hi = (0, NB) if half == 0 else (NB, DIM)
                py = psU.tile([P, NB], F32, tag="u", name=f"tpy{s}_{half}")
                for k in range(MQ):
                    nc.tensor.matmul(
                        py[:, 0:hi - lo],
                        lhsT=ot_sb[:, k, s * P:(s + 1) * P],
                        rhs=wo_sb[:, k, lo:hi],
                        start=(k == 0),
                        stop=(k == MQ - 1),
                    )
                if half == 0:
                    out_proj_unit.y[s] = yout.tile([P, DIM], F32, tag="y",
                                                   name=f"y{s}")
                y_sb = out_proj_unit.y[s]
                if half == 0:
                    nc.vector.tensor_copy(y_sb[:, lo:hi], py[:, 0:hi - lo])
                else:
                    nc.scalar.copy(y_sb[:, lo:hi], py[:, 0:hi - lo])
                    nc.sync.dma_start(out[s * P:(s + 1) * P, :], y_sb[:])

    if split_waits:
        _split_multi_waits(nc)
    return nc


_NC = None


def _get_nc():
    global _NC
    if _NC is None:
        _NC = build_nc()
    return _NC


def _bf(a):
    return np.ascontiguousarray(np.asarray(a, dtype=np.float32)).astype(NPBF16)


def make_in_maps(query, key, value, wq, bq, wk, bk, wv, bv, wo, bo):
    query = np.asarray(query, np.float32)
    key = np.asarray(key, np.float32)
    value = np.asarray(value, np.float32)
    wq = np.asarray(wq, np.float32)
    wk = np.asarray(wk, np.float32)
    wv = np.asarray(wv, np.float32)
    wo = np.asarray(wo, np.float32)
    in_maps = []
    for b in range(B):
        xqT = _bf(query[b].T)
        xkT = _bf(key[b].T)
        xvT = _bf(value[b].T)
        for g in range(GROUPS):
            sl = slice(g * GE, (g + 1) * GE)
            in_maps.append({
                "xqT": xqT,
                "xkT": xkT,
                "xvT": xvT,
                "wq": _bf(wq[:, sl]),
                "wk": _bf(wk[:, sl]),
                "wv": _bf(wv[:, sl]),
                "wo": _bf(wo[sl, :]),
                "bq": np.ascontiguousarray(np.asarray(bq, np.float32)[sl]),
            })
    return in_maps


def kernel(query, key, value, wq, bq, wk, bk, wv, bv, wo, bo, **kw):
    nc = _get_nc()
    in_maps = make_in_maps(query, key, value, wq, bq, wk, bk, wv, bv, wo, bo)
    res = run_bass_kernel_spmd(nc, in_maps, list(range(NCORES))).results
    # bv is dropped on device (softmax rows sum to 1) and folded here;
    # bk shifts scores by a per-query constant and is softmax-invariant.
    fold = (np.asarray(bv, np.float32) @ np.asarray(wo, np.float32)
            + np.asarray(bo, np.float32))
    out = np.empty((B, S, DIM), np.float32)
    for b in range(B):
        out[b] = res[b * GROUPS]["out"] + res[b * GROUPS + 1]["out"] + fold
    return out


# revision 38
# speedup vs baseline: 1.0578x; 1.0027x over previous
"""Trainium2 Bass kernel for nn_MultiHeadAttention (B=4, S=2048, DIM=768,
EMBED=512, HEADS=8, HEAD_DIM=64), distributed over 8 NeuronCores.

Sharding: core (b, g) with b in 0..3 (batch, data parallel) and g in 0..1
(head-group of 4 heads, tensor parallel). Each core computes a partial
output Y_partial[b,g] = softmax(QK^T/8) V @ Wo[g-slice]; the host sums the
two group partials per batch and adds the fused output bias bv@wo + bo
(bv is removable on device because softmax rows sum to 1; bk only shifts
each softmax row by a per-query constant, so it is dropped entirely).

Device dataflow per core (bf16 matmuls, fp32 PSUM accumulation):
  - host supplies x^T (DIM on partitions) so no on-device transposes exist
  - Q^T = Wq^T x^T + bq, K^T = Wk^T x^T (no bias) -> [256, S]
  - V packed as [V_h | ones] / [ones | V_h] per local head so the PV
    matmul also emits the softmax denominator replicated 64x
  - S^T via lhsT=K^T chunk, rhs=Q^T block; two heads run as concurrent
    K=64 PE tiles (row packing)
  - exp on ScalarE ACTIVATE(Exp, scale=1/8) straight out of PSUM, FD=1024;
    the first N_DVE chunks of each block run on VectorE instead via a
    Schraudolph bit-exp (int16(s*A+B) bitcast to bf16, ~2-4% sawtooth
    that mostly cancels in the softmax ratio)
  - 1/rowsum via 2-step Newton from a constant seed on GpSimd (all-SBUF),
    keeping VectorE free for PSUM evacuations
  - O^T.T @ Wo -> [S, DIM], evac + DMA per s-chunk
Scheduling: activations stream in [128,512] pieces split across both DMA
rings in consumption order (xk-n0, xq-n0, xk-n1, xv-n0, xv-n1, xk-n2/3,
xv-n2/3, xq-n1..3) so the first exp fires ~8us in; K/V projections and the
block-0 PV backlog (V lands late) drain through a per-chunk filler schedule
inside the attention loops so the PE stream never head-blocks on a DMA.
A post-pass splits multi-semaphore waits and the gpsimd RANGE_CLEAR into
single-wait NoOps for this image's stricter walrus.
"""

import numpy as np
import ml_dtypes

import concourse.bass as bass
import concourse.tile as tile
from concourse import mybir
from concourse.bass_utils import run_bass_kernel_spmd

BF16 = mybir.dt.bfloat16
F32 = mybir.dt.float32
I16 = mybir.dt.int16
NPBF16 = ml_dtypes.bfloat16

B, S, DIM, EMBED, HEADS, HEAD_DIM = 4, 2048, 768, 512, 8, 64
P = 128
KD = DIM // P          # 6   contraction chunks for projections
GROUPS = 2             # head-groups (tensor-parallel split)
GE = EMBED // GROUPS   # 256 embed columns per group
GH = HEADS // GROUPS   # 4   heads per group
MQ = GE // P           # 2   e-chunks per group
SC = S // P            # 16  sequence chunks of 128
NB = 512               # matmul free-dim block
NQ = S // NB           # 4   query blocks
SCALE = 0.125          # 1/sqrt(HEAD_DIM)
NCORES = B * GROUPS    # 8
X0 = 1.0 / 2146.0      # Newton seed for 1/rowsum

# Schraudolph bit-exp on DVE for the first N_DVE chunks of each block:
# bf16 bits of exp(s*SCALE) ~ round(s * 128*log2(e)*SCALE + 127*128 - sig)
N_DVE = 2
SCH_A = 128.0 * 1.4426950408889634 * SCALE
SCH_B = 127.0 * 128.0 - 0.5 * 128.0 * 0.043

ESP_BUFS = 16          # es backlog depth (chunks) for the deferred-PV ramp
PV_DRAIN_FROM = 13     # global chunk index where block-0 PV drains start


def _split_multi_waits(nc):
    """The walrus build in this image accepts at most ONE sem-wait per
    instruction (setupSyncWait: 'Too many sync wait commands'), while Tile
    freely attaches several.  Hoist all but the last wait of each
    instruction onto same-engine NoOps inserted immediately before it —
    identical blocking semantics, one wait per instruction."""
    ctr = 0
    for f in nc.m.functions:
        for blk in f.blocks:
            il = blk.instructions
            out = []
            for inst in il:
                if type(inst).__name__ == "InstISA":
                    # kernel-tail gpsimd.sem_clear (RANGE_CLEAR): this
                    # walrus rejects its encoding ("ISA wrong length").
                    # NRT re-initializes semaphore state per execution, so
                    # replace it with a NoOp carrying the same syncs.
                    nop = mybir.InstNoOp(
                        name=f"{inst.name}-isanop", ins=[], outs=[]
                    )
                    nop.engine = inst.engine
                    nop.sync_info = inst.sync_info
                    out.append(nop)
                    continue
                si = inst.sync_info
                if si is not None and si.on_wait and len(si.on_wait) > 1:
                    waits = list(si.on_wait)
                    for w in waits[:-1]:
                        ctr += 1
                        nop = mybir.InstNoOp(
                            name=f"I-waitsplit-{ctr}", ins=[], outs=[]
                        )
                        nop.engine = inst.engine
                        nop.sync_info = mybir.SyncInfo(on_wait=[w], on_update=[])
                        out.append(nop)
                    si.on_wait = [waits[-1]]
                out.append(inst)
            il[:] = out
    return ctr


def build_nc(split_waits=True):
    nc = bass.Bass("TRN2", target_bir_lowering=False, debug=False)

    xqT = nc.dram_tensor("xqT", [DIM, S], BF16, kind="ExternalInput").ap()
    xkT = nc.dram_tensor("xkT", [DIM, S], BF16, kind="ExternalInput").ap()
    xvT = nc.dram_tensor("xvT", [DIM, S], BF16, kind="ExternalInput").ap()
    wq = nc.dram_tensor("wq", [DIM, GE], BF16, kind="ExternalInput").ap()
    wk = nc.dram_tensor("wk", [DIM, GE], BF16, kind="ExternalInput").ap()
    wv = nc.dram_tensor("wv", [DIM, GE], BF16, kind="ExternalInput").ap()
    wo = nc.dram_tensor("wo", [GE, DIM], BF16, kind="ExternalInput").ap()
    bq = nc.dram_tensor("bq", [GE], F32, kind="ExternalInput").ap()
    out = nc.dram_tensor("out", [S, DIM], F32, kind="ExternalOutput").ap()

    add = mybir.AluOpType.add
    mult = mybir.AluOpType.mult
    Exp = mybir.ActivationFunctionType.Exp

    with tile.TileContext(nc) as tc:
        with (
            tc.tile_pool(name="const", bufs=1) as const,
            # PSUM: "s" = 2 slots x [P,2,NB] (score pairs, 4 banks);
            #       "u" = 4 slots x 1 bank (proj blocks, PV accumulators,
            #             next-q proj, out-proj halves) = 8 banks total.
            tc.tile_pool(name="psS", bufs=2, space="PSUM") as psS,
            tc.tile_pool(name="psU", bufs=4, space="PSUM") as psU,
            tc.tile_pool(name="esp", bufs=ESP_BUFS) as esp,
            tc.tile_pool(name="nrm", bufs=2) as nrm,
            tc.tile_pool(name="yout", bufs=3) as yout,
            tc.tile_pool(name="xin", bufs=3) as xin,
        ):
            wq_sb = const.tile([P, KD, GE], BF16, tag="wq")
            wk_sb = const.tile([P, KD, GE], BF16, tag="wk")
            wv_sb = const.tile([P, KD, GE], BF16, tag="wv")
            wo_sb = const.tile([P, MQ, DIM], BF16, tag="wo")
            bq_sb = const.tile([P, MQ], F32, tag="bq")
            qt_sb = const.tile([P, MQ, S], BF16, tag="qt")   # Q^T
            kt_sb = const.tile([P, MQ, S], BF16, tag="kt")   # K^T
            ot_sb = const.tile([P, MQ, S], BF16, tag="ot")   # O^T
            # V in PV-lhsT layout: per (s-chunk, head) a [128, 128] block
            # of [V_h | ones] (even local head) or [ones | V_h] (odd); the
            # ones columns make the PV matmul also produce the softmax
            # denominator (replicated 64x), partition-aligned with the
            # other head's slot.
            v_sb = const.tile([P, SC, GH, P], BF16, tag="v")
            nc.vector.memset(v_sb[:], 1.0)

            # small ones tile: PE warm-up operand (HAM un-throttles after
            # ~3.4us of sustained matmul activity; warming on dummies while
            # the first DMAs stream means the real prelude runs at 2.4GHz).
            # memset on gpsimd: the DVE is busy with the big v_sb memset.
            dum_sb = const.tile([P, NB], BF16, tag="dum")
            nc.gpsimd.memset(dum_sb[:], 1.0)

            # --- DMAs: sync ring carries weights + xq n-block 0 (so the
            # Q0 projection starts early); gpsimd ring carries the rest in
            # [128,1024] halves (DMA triggers cost ~0.64us each, so pieces
            # stay coarse), ordered by first consumption.
            xk_sb = xin.tile([P, KD, S], BF16, tag="x", name="xk")
            xq_sb = xin.tile([P, KD, S], BF16, tag="x", name="xq")
            xv_sb = xin.tile([P, KD, S], BF16, tag="x", name="xv")

            nc.sync.dma_start(wq_sb[:], wq.rearrange("(k p) e -> p k e", p=P))
            nc.sync.dma_start(bq_sb[:], bq.rearrange("(m p) -> p m", p=P))
            nc.sync.dma_start(wk_sb[:], wk.rearrange("(k p) e -> p k e", p=P))
            nc.sync.dma_start(wv_sb[:], wv.rearrange("(k p) e -> p k e", p=P))
            nc.sync.dma_start(wo_sb[:], wo.rearrange("(m p) d -> p m d", p=P))

            for k in range(KD):     # xq n-block 0 first: Q0-proj unblocks
                nc.gpsimd.dma_start(xq_sb[:, k, 0:NB], xqT[k * P:(k + 1) * P, 0:NB])
            for k in range(KD):     # xk (K-proj, consumed through block 0)
                nc.gpsimd.dma_start(xk_sb[:, k, :], xkT[k * P:(k + 1) * P, :])
            for k in range(KD):     # xv (V-proj, fillers from mid-block 0)
                nc.gpsimd.dma_start(xv_sb[:, k, :], xvT[k * P:(k + 1) * P, :])
            for k in range(KD):     # xq n-blocks 1-3 (q1 proj is in block 1)
                nc.gpsimd.dma_start(xq_sb[:, k, NB:S], xqT[k * P:(k + 1) * P, NB:S])

            # PE warm-up: ~4us of dummy matmuls while the DMAs stream
            wu = psU.tile([P, NB], F32, tag="u", name="warmup")
            for i in range(9):
                nc.tensor.matmul(wu[:], lhsT=dum_sb[:, 0:P], rhs=dum_sb[:],
                                 start=(i == 0), stop=(i == 8))
            nc.vector.tensor_copy(dum_sb[0:1, 0:4], wu[0:1, 0:4])

            proj_ps = {}

            def qk_proj_half(x_sb, w_sb, b_sb, dst, m, n, half):
                """3 of the 6 contraction matmuls of a projection block;
                half==1 finishes and evacuates.  Split so fillers stay
                ~0.7us and never pile up in the PE stream."""
                key = (dst.name, m, n)
                if half == 0:
                    proj_ps[key] = psU.tile([P, NB], F32, tag="u",
                                            name=f"pj{dst.name}_{m}_{n}")
                ps = proj_ps[key]
                for k in range(3 * half, 3 * half + 3):
                    nc.tensor.matmul(
                        ps[:],
                        lhsT=w_sb[:, k, m * P:(m + 1) * P],
                        rhs=x_sb[:, k, n * NB:(n + 1) * NB],
                        start=(k == 0),
                        stop=(k == KD - 1),
                    )
                if half == 0:
                    return
                del proj_ps[key]
                if b_sb is None:
                    nc.vector.tensor_copy(dst[:, m, n * NB:(n + 1) * NB], ps[:])
                else:
                    nc.vector.tensor_scalar(
                        out=dst[:, m, n * NB:(n + 1) * NB],
                        in0=ps[:],
                        scalar1=b_sb[:, m:m + 1],
                        scalar2=None,
                        op0=add,
                    )

            def qk_proj_block(x_sb, w_sb, b_sb, dst, m, n):
                qk_proj_half(x_sb, w_sb, b_sb, dst, m, n, 0)
                qk_proj_half(x_sb, w_sb, b_sb, dst, m, n, 1)

            def v_proj_chunk(s):
                ps = psU.tile([P, GE], F32, tag="u", name=f"pv{s}")
                for k in range(KD):
                    nc.tensor.matmul(
                        ps[:],
                        lhsT=xv_sb[:, k, s * P:(s + 1) * P],
                        rhs=wv_sb[:, k, :],
                        start=(k == 0),
                        stop=(k == KD - 1),
                    )
                ps_h = ps.rearrange("p (h d) -> p h d", d=HEAD_DIM)
                # even local heads -> cols [0:64], odd -> cols [64:128]
                nc.vector.tensor_copy(
                    v_sb[:, s, 0::2, 0:HEAD_DIM], ps_h[:, 0::2, :]
                )
                nc.vector.tensor_copy(
                    v_sb[:, s, 1::2, HEAD_DIM:P], ps_h[:, 1::2, :]
                )

            # out-projection in half-s-chunk units (2 matmuls + 1 copy)
            def out_proj_unit(s, half):
                lo, hi = (0, NB) if half == 0 else (NB, DIM)
                py = psU.tile([P, NB], F32, tag="u", name=f"py{s}_{half}")
                for k in range(MQ):
                    nc.tensor.matmul(
                        py[:, 0:hi - lo],
                        lhsT=ot_sb[:, k, s * P:(s + 1) * P],
                        rhs=wo_sb[:, k, lo:hi],
                        start=(k == 0),
                        stop=(k == MQ - 1),
                    )
                if half == 0:
                    out_proj_unit.y[s] = yout.tile([P, DIM], F32, tag="y",
                                                   name=f"y{s}")
                y_sb = out_proj_unit.y[s]
                nc.vector.tensor_copy(y_sb[:, lo:hi], py[:, 0:hi - lo])
                if half == 1:
                    nc.sync.dma_start(out[s * P:(s + 1) * P, :], y_sb[:])
            out_proj_unit.y = {}
            out_proj_unit.todo = 0
            out_proj_unit.avail = 0

            def drain_out_proj():
                if out_proj_unit.todo < out_proj_unit.avail:
                    unit = out_proj_unit.todo
                    out_proj_unit(unit // 2, unit % 2)
                    out_proj_unit.todo = unit + 1

            def make_normalize(pu, hp, q):
                """1/rowsum via 2-step Newton from a constant seed (~1e-6).
                Head j=0's chain on VectorE, j=1's on GpSimd, emitted
                interleaved so both run concurrently and the recip-DMA
                waits never head-block the ur copies."""
                def _norm():
                    ur, x1, tmp, eng, rsl, usl = {}, {}, {}, {}, {}, {}
                    for j in range(2):
                        eng[j] = nc.vector if j == 0 else nc.gpsimd
                        usl[j] = slice(j * HEAD_DIM, (j + 1) * HEAD_DIM)
                        rsl[j] = slice((1 - j) * HEAD_DIM, (2 - j) * HEAD_DIM)
                        ur[j] = nrm.tile([P, NB], F32, tag=f"ur{j}",
                                         name=f"ur{hp}_{q}_{j}")
                        # full copy: releases the PV bank early
                        nc.vector.tensor_copy(ur[j][:], pu[j][:])
                        x1[j] = nrm.tile([P, NB], F32, tag=f"x1{j}",
                                         name=f"x1{hp}_{q}_{j}")
                        tmp[j] = nrm.tile([P, NB], F32, tag=f"tmp{j}",
                                          name=f"tmp{hp}_{q}_{j}")
                    for j in range(2):
                        eng[j].tensor_scalar(    # x1 = 2x0 - x0^2 r
                            out=x1[j][rsl[j], :], in0=ur[j][rsl[j], :],
                            scalar1=-X0 * X0, scalar2=2.0 * X0,
                            op0=mult, op1=add,
                        )
                    for j in range(2):
                        eng[j].tensor_tensor(    # e = r * x1
                            out=tmp[j][rsl[j], :], in0=ur[j][rsl[j], :],
                            in1=x1[j][rsl[j], :], op=mult,
                        )
                        eng[j].tensor_scalar(    # u = 2 - e
                            out=tmp[j][rsl[j], :], in0=tmp[j][rsl[j], :],
                            scalar1=-1.0, scalar2=2.0,
                            op0=mult, op1=add,
                        )
                        eng[j].tensor_tensor(    # x2 = x1 * u
                            out=x1[j][rsl[j], :], in0=x1[j][rsl[j], :],
                            in1=tmp[j][rsl[j], :], op=mult,
                        )
                    # recip rows onto U partitions, then scale into O^T
                    nc.sync.dma_start(x1[0][usl[0], :], x1[0][rsl[0], :])
                    nc.gpsimd.dma_start(x1[1][usl[1], :], x1[1][rsl[1], :])
                    for j in range(2):
                        nc.vector.tensor_tensor(
                            out=ot_sb[usl[j], hp, q * NB:(q + 1) * NB],
                            in0=ur[j][usl[j], :],
                            in1=x1[j][usl[j], :],
                            op=mult,
                        )
                return _norm

            # ---------- per-chunk filler schedule ----------
            # sched[gc] -> PE-work closures drained at global chunk gc
            # (gc = 16*(2q+hp) + m; chunk gc runs ~ 8 + 1.1*gc us).  Each
            # closure's data deps are landed (or land within ~1us) by the
            # time the PE reaches it, so the in-order PE stream never
            # head-blocks.
            sched = {}

            def at(gc, fn):
                sched.setdefault(gc, []).append(fn)

            # Remaining projections ride blocks 0-1 as halves (~0.7us
            # fillers): hp0's K-proj n1-n3 first (consumed within block 0),
            # hp1's K-proj + Q0-proj e-chunk 1 (consumed from block 1),
            # V-proj paced 1/chunk behind the xv halves.
            def KPH(m, n, half):
                return lambda: qk_proj_half(xk_sb, wk_sb, None, kt_sb, m, n, half)

            def QPH(m, n, half):
                return lambda: qk_proj_half(xq_sb, wq_sb, bq_sb, qt_sb, m, n, half)
            at(0, KPH(0, 1, 0))
            at(1, KPH(0, 1, 1))
            at(2, KPH(0, 2, 0))
            at(3, KPH(0, 2, 1))
            at(4, KPH(0, 3, 0))
            at(5, KPH(0, 3, 1))
            at(6, KPH(1, 0, 0))
            at(7, KPH(1, 0, 1))
            at(8, QPH(1, 0, 0))
            at(9, QPH(1, 0, 1))
            at(10, KPH(1, 1, 0))
            at(11, KPH(1, 1, 1))
            at(12, KPH(1, 2, 0))
            at(13, KPH(1, 2, 1))
            at(14, KPH(1, 3, 0))
            at(15, KPH(1, 3, 1))
            # all V-proj inside block 0 (its fillers may allocate PSUM
            # slots; from block 1 all four slots hold PV accumulators)
            VP_SLOTS = [7, 8, 9, 10, 11, 11, 12, 12, 13,
                        13, 14, 14, 14, 15, 15, 15]
            for s in range(SC):
                at(VP_SLOTS[s], lambda s=s: v_proj_chunk(s))

            pv_queue = []            # deferred PV chunk closures, FIFO
            qk_partial = {}

            def q_phase(qn, mq_idx, phase):
                if phase == 0:
                    qp = psU.tile([P, NB], F32, tag="u", name=f"qp{qn}_{mq_idx}")
                    qk_partial[mq_idx] = qp
                qp = qk_partial[mq_idx]
                for k in (2 * phase, 2 * phase + 1):
                    nc.tensor.matmul(
                        qp[:],
                        lhsT=wq_sb[:, k, mq_idx * P:(mq_idx + 1) * P],
                        rhs=xq_sb[:, k, qn * NB:(qn + 1) * NB],
                        start=(k == 0),
                        stop=(k == KD - 1),
                    )
                if phase == 2:
                    nc.vector.tensor_scalar(
                        out=qt_sb[:, mq_idx, qn * NB:(qn + 1) * NB],
                        in0=qp[:],
                        scalar1=bq_sb[:, mq_idx:mq_idx + 1],
                        scalar2=None,
                        op0=add,
                    )

            # ---- prelude: Q0 first (its DMA lands first), then e-chunk 0
            # of K n-block 0; the rest rides the block-0 fillers ----
            qk_proj_block(xq_sb, wq_sb, bq_sb, qt_sb, 0, 0)
            qk_proj_block(xk_sb, wk_sb, None, kt_sb, 0, 0)

            pend = []

            # ---- attention, one (q, head-pair) block at a time ----
            for q in range(NQ):
                for hp in range(MQ):
                    bi = 2 * q + hp
                    pu = [
                        psU.tile([P, NB], F32, tag="u",
                                 name=f"pu{hp}_{q}_{j}")
                        for j in range(2)
                    ]
                    for m in range(SC):
                        gc = 16 * bi + m
                        # previous block's deferred normalize once this
                        # block is under way and its accumulators are
                        # complete (the PV backlog has fully drained)
                        if m >= 2 and pend and not pv_queue:
                            pend.pop(0)()
                        # fillers + backlog drains BEFORE this chunk's QK:
                        # the QK blocks the in-order PE stream on a psS
                        # slot (exp-paced), so queued work must precede it
                        if m >= 2:
                            for fn in sched.pop(gc, ()):
                                fn()
                            if pv_queue and gc >= PV_DRAIN_FROM:
                                budget = 1 if bi == 0 else 3
                                for _ in range(budget):
                                    if pv_queue:
                                        pv_queue.pop(0)()
                        ss = psS.tile([P, 2, NB], F32, tag="s")
                        for j in range(2):
                            lo, hi = j * HEAD_DIM, (j + 1) * HEAD_DIM
                            nc.tensor.matmul(
                                ss[:, j, :],
                                lhsT=kt_sb[lo:hi, hp, m * P:(m + 1) * P],
                                rhs=qt_sb[lo:hi, hp, q * NB:(q + 1) * NB],
                                start=True,
                                stop=True,
                            )
                        if m < N_DVE:
                            # Schraudolph bit-exp on DVE: bf16 bits via
                            # int16 convert of s*A+B, written into a bf16
                            # tile through a bitcast view
                            es = esp.tile([P, 2, NB], BF16, tag="es",
                                          name=f"es{bi}_{m}")
                            nc.vector.tensor_scalar(
                                out=es[:].bitcast(I16),
                                in0=ss[:],
                                scalar1=SCH_A, scalar2=SCH_B,
                                op0=mult, op1=add,
                            )
                        else:
                            es = esp.tile([P, 2, NB], BF16, tag="es",
                                          name=f"es{bi}_{m}")
                            nc.scalar.activation(es[:], ss[:], Exp, scale=SCALE)

                        def pv(pu=pu, hp=hp, m=m, es=es):
                            for j in range(2):
                                nc.tensor.matmul(
                                    pu[j][:],
                                    lhsT=v_sb[:, m, 2 * hp + j, :],
                                    rhs=es[:, j, :],
                                    start=(m == 0),
                                    stop=(m == SC - 1),
                                )
                        # PV runs inline once the backlog is clear (from
                        # mid-block-1 on); blocks 0/1 defer through the
                        # queue while V-proj catches up with the xv DMAs
                        if pv_queue or bi == 0:
                            pv_queue.append(pv)
                        else:
                            pv()
                        # m<2 fillers/drains go after the QK (no
                        # head-blocking of the block's first scores)
                        if m < 2:
                            for fn in sched.pop(gc, ()):
                                fn()
                            if pv_queue and gc >= PV_DRAIN_FROM:
                                budget = 1 if bi == 0 else 3
                                for _ in range(budget):
                                    if pv_queue:
                                        pv_queue.pop(0)()
                        # out-proj half-units: 2 late in hp0 blocks (the
                        # previous normalize's ~7us chain gates ot), 6
                        # spread through hp1 blocks
                        if (hp == 0 and m in (12, 14)) or \
                           (hp == 1 and m in (0, 2, 4, 6, 8, 10)):
                            drain_out_proj()
                        # next q block's Q^T projection, 2 matmuls a time,
                        # finishing 3 chunks before the next block uses
                        # qt (block 1 holds no free PSUM slot before its
                        # backlog normalize at ~m10)
                        q_slots = (10, 11, 12, 13, 14, 15) if bi == 1 else (8, 9, 10, 11, 12, 13)
                        if hp == 1 and q + 1 < NQ and m in q_slots:
                            ph = m - q_slots[0]
                            mq_idx, phase = divmod(ph, 3)
                            q_phase(q + 1, mq_idx, phase)
                    # safety: by design the queue is empty from block 2 on
                    if bi >= 2:
                        while pv_queue:
                            pv_queue.pop(0)()
                    last = (q == NQ - 1 and hp == MQ - 1)
                    nrm_fn = make_normalize(pu, hp, q)
                    if last:
                        nrm_fn()
                    else:
                        pend.append(nrm_fn)
                    if hp == 1:
                        out_proj_unit.avail = 8 * (q + 1)
            for th in pend:
                th()
            # ---- tail: the last q-block's out-projection; evacuations
            # alternate VectorE/ScalarE (ScalarE is idle by now) so the
            # PSUM drain doesn't serialize on one engine ----
            for unit in range(out_proj_unit.todo, 8 * NQ):
                s, half = unit // 2, unit % 2
                lo, hi = (0, NB) if half == 0 else (NB, DIM)
                py = psU.tile([P, NB], F32, tag="u", name=f"tpy{s}_{half}")
                for k in range(MQ):
                    nc.tensor.matmul(
                        py[:, 0:hi - lo],
                        lhsT=ot_sb[:, k, s * P:(s + 1) * P],
                        rhs=wo_sb[:, k, lo:hi],
                        start=(k == 0),
                        stop=(k == MQ - 1),
                    )
                if half == 0:
                    out_proj_unit.y[s] = yout.tile([P, DIM], F32, tag="y",
                                                   name=f"y{s}")
                y_sb = out_proj_unit.y[s]
                if half == 0:
                    nc.vector.tensor_copy(y_sb[:, lo:hi], py[:, 0:hi - lo])
                else:
                    nc.scalar.copy(y_sb[:, lo:hi], py[:, 0:hi - lo])
                    nc.sync.dma_start(out[s * P:(s + 1) * P, :], y_sb[:])

    if split_waits:
        _split_multi_waits(nc)
    return nc


_NC = None


def _get_nc():
    global _NC
    if _NC is None:
        _NC = build_nc()
    return _NC


def _bf(a):
    return np.ascontiguousarray(np.asarray(a, dtype=np.float32)).astype(NPBF16)


def make_in_maps(query, key, value, wq, bq, wk, bk, wv, bv, wo, bo):
    query = np.asarray(query, np.float32)
    key = np.asarray(key, np.float32)
    value = np.asarray(value, np.float32)
    wq = np.asarray(wq, np.float32)
    wk = np.asarray(wk, np.float32)
    wv = np.asarray(wv, np.float32)
    wo = np.asarray(wo, np.float32)
    in_maps = []
    for b in range(B):
        xqT = _bf(query[b].T)
        xkT = _bf(key[b].T)
        xvT = _bf(value[b].T)
        for g in range(GROUPS):
            sl = slice(g * GE, (g + 1) * GE)
            in_maps.append({
                "xqT": xqT,
                "xkT": xkT,
                "xvT": xvT,
                "wq": _bf(wq[:, sl]),
                "wk": _bf(wk[:, sl]),
                "wv": _bf(wv[:, sl]),
                "wo": _bf(wo[sl, :]),
                "bq": np.ascontiguousarray(np.asarray(bq, np.float32)[sl]),
            })
    return in_maps


def kernel(query, key, value, wq, bq, wk, bk, wv, bv, wo, bo, **kw):
    nc = _get_nc()
    in_maps = make_in_maps(query, key, value, wq, bq, wk, bk, wv, bv, wo, bo)
    res = run_bass_kernel_spmd(nc, in_maps, list(range(NCORES))).results
    # bv is dropped on device (softmax rows sum to 1) and folded here;
    # bk shifts scores by a per-query constant and is softmax-invariant.
    fold = (np.asarray(bv, np.float32) @ np.asarray(wo, np.float32)
            + np.asarray(bo, np.float32))
    out = np.empty((B, S, DIM), np.float32)
    for b in range(B):
        out[b] = res[b * GROUPS]["out"] + res[b * GROUPS + 1]["out"] + fold
    return out
